# revision 1
# baseline (speedup 1.0000x reference)
"""Self-contained Trainium2 Bass kernel for a 12-head attention layer.

Problem: x[4,2048,768] -> attention(QKV projections, softmax, context),
NUM_HEADS=12, SIZE_PER_HEAD=64, additive mask from mask[4,2048].

Sharding over 8 NeuronCores: core c handles batch b=c//2 and head-group
hg=c%2 (6 heads, 384 feature columns).  Everything is local per core:
no collectives.  Host-side prep per core:
  - xT_aug [769,2048] bf16 = concat(x[b].T, ones-row)   (bias via matmul)
  - wq [769,384] bf16 = concat(Wq[:,cols]/8, bq[cols]/8)  (1/sqrt(64) folded)
  - wk [769,384] bf16 = concat(Wk[:,cols], bk[cols])
  - wv [769,390] bf16: head-major with a 65th "ones" column per head so the
    context matmul also produces the softmax denominator (row 64 of ctx').
  - adder [128,16] f32 = (mask[b]-1)*10000 laid out per T-tile (ACT bias).

On-chip per core (all matmuls bf16, PSUM f32):
  QT,KT [384,2048] = W.T @ xT_aug      (feature-major projections)
  V'    [2048,390] = xT_aug.T @ wv     (token-major, with ones cols)
  per head h, F-block of 1024, T-tile of 128:
    S^T[T,F] psum  = KT_h^T @ QT_h     (K=64 contraction)
    exp sbuf bf16  = ACT Exp(S^T + adder[T])   (mask as per-partition bias)
    ctx'[65,F] psum += V'_h[T-tile].T @ exp    (K=128; row 64 = denominator)
  normalize per segment, fully off the TensorEngine: reciprocal of
  denominators (DVE, [128,*] layout via sbuf-sbuf DMA gather/scatter),
  gpsimd partition_broadcast of the reciprocal row, DVE multiply, DMA out.

Output per core: ctx^T [384,2048] f32; host transposes/concats to [4,2048,768].
"""

import numpy as np
import ml_dtypes

B, S, D = 4, 2048, 768
H, DH = 12, 64
HL = 6          # heads per core
DL = HL * DH    # 384 feature columns per core
NCORES = 8
P = 128
KO = 6          # full k-subtiles of the 768 contraction
NT = S // P     # 16 T-tiles
FB = 1024       # F block size
NFB = S // FB   # 2 F blocks
NSUB = FB // 512  # 512-wide matmul chunks per F block

_CACHE = {}


def _build(with_bias=False):
    import concourse.mybir as mybir
    import concourse.tile as tile
    from concourse import bacc

    dt = mybir.dt
    Exp = mybir.ActivationFunctionType.Exp
    Alu = mybir.AluOpType

    nc = bacc.Bacc("TRN2", target_bir_lowering=False, debug=False,
                   num_devices=NCORES)

    xT = nc.dram_tensor("xT", [D + 1, S], dt.bfloat16, kind="ExternalInput")
    wq = nc.dram_tensor("wq", [D + 1, DL], dt.bfloat16, kind="ExternalInput")
    wk = nc.dram_tensor("wk", [D + 1, DL], dt.bfloat16, kind="ExternalInput")
    wv = nc.dram_tensor("wv", [D + 1, HL * (DH + 1)], dt.bfloat16,
                        kind="ExternalInput")
    adder = nc.dram_tensor("adder", [P, NT], dt.float32, kind="ExternalInput")
    out = nc.dram_tensor("out", [DL, S], dt.float32, kind="ExternalOutput")

    with tile.TileContext(nc) as tc:
        with (
            tc.tile_pool(name="persist", bufs=1) as sb,
            tc.tile_pool(name="work", bufs=4) as work,
            tc.tile_pool(name="fin", bufs=3) as fin,
            tc.tile_pool(name="ps_s", bufs=2, space="PSUM") as ps_s,
            tc.tile_pool(name="ps_c", bufs=2, space="PSUM") as ps_c,
        ):
            # ---- input DMA (priority order: xT, pair-0 weights first) ----
            xTs = sb.tile([P, KO + 1, S], dt.bfloat16, tag="xTs")
            for ko in range(KO):
                nc.sync.dma_start(
                    xTs[:, ko, :], xT.ap()[ko * P:(ko + 1) * P, :])
            nc.sync.dma_start(xTs[0:1, KO, :], xT.ap()[D:D + 1, :])

            wqs = sb.tile([P, KO + 1, DL], dt.bfloat16, tag="wqs")
            wks = sb.tile([P, KO + 1, DL], dt.bfloat16, tag="wks")
            wvs = sb.tile([P, KO + 1, HL * (DH + 1)], dt.bfloat16, tag="wvs")
            adder_sb = sb.tile([P, NT], dt.float32, tag="adder")

            def dma_w_mtile(w_dram, w_sb, c0, c1):
                nc.sync.dma_start(
                    w_sb[:, 0:KO, c0:c1],
                    w_dram.ap()[0:D, c0:c1].rearrange(
                        "(ko p) m -> p ko m", p=P))
                nc.sync.dma_start(w_sb[0:1, KO, c0:c1],
                                  w_dram.ap()[D:D + 1, c0:c1])

            dma_w_mtile(wq, wqs, 0, P)
            dma_w_mtile(wk, wks, 0, P)
            dma_w_mtile(wv, wvs, 0, 2 * (DH + 1))
            nc.sync.dma_start(adder_sb[:], adder.ap())
            dma_w_mtile(wq, wqs, P, DL)
            dma_w_mtile(wk, wks, P, DL)
            dma_w_mtile(wv, wvs, 2 * (DH + 1), HL * (DH + 1))

            # persistent projection outputs
            qt = sb.tile([P, 3, S], dt.bfloat16, tag="qt")   # Q^T/8 (+bias)
            kt = sb.tile([P, 3, S], dt.bfloat16, tag="kt")   # K^T (+bias)
            vp = sb.tile([P, NT, HL * (DH + 1)], dt.bfloat16, tag="vp")
            # unnormalized ctx' (65 rows per head; row 64 = denominator)
            ctxs = sb.tile([DH + 1, HL, S], dt.float32, tag="ctxs")

            KE = KO + 1 if with_bias else KO  # k-subtiles incl. optional bias

            # ---- projections (psum from ps_c: free at prefix/boundaries,
            # decoupled from the scores/exp slot rotation) ----
            def proj_qk(w_sb, dst, m, ns=(0, 1, 2, 3)):
                for n in ns:
                    pt = ps_c.tile([P, 512], dt.float32, tag="c", name="pt")
                    for k in range(KE):
                        lhsT = (w_sb[:, k, m * P:(m + 1) * P] if k < KO
                                else w_sb[0:1, k, m * P:(m + 1) * P])
                        rhs = (xTs[:, k, n * 512:(n + 1) * 512] if k < KO
                               else xTs[0:1, k, n * 512:(n + 1) * 512])
                        nc.tensor.matmul(pt[:], lhsT, rhs,
                                         start=(k == 0), stop=(k == KE - 1))
                    nc.vector.tensor_copy(dst[:, m, n * 512:(n + 1) * 512],
                                          pt[:])

            def proj_v(mt):
                # full-width V' tile (all 6 heads at once: N=390 amortizes
                # LDWEIGHTS much better than per-pair N=130 chunks).
                # Always includes the k=768 row: it carries the ones
                # indicator that builds V's 65th (denominator) column.
                w = HL * (DH + 1)
                pt = ps_c.tile([P, 512], dt.float32, tag="c", name="pt")
                for k in range(KO + 1):
                    lhsT = (xTs[:, k, mt * P:(mt + 1) * P] if k < KO
                            else xTs[0:1, k, mt * P:(mt + 1) * P])
                    rhs = wvs[:, k, :] if k < KO else wvs[0:1, k, :]
                    nc.tensor.matmul(pt[:, :w], lhsT, rhs,
                                     start=(k == 0), stop=(k == KO))
                nc.vector.tensor_copy(vp[:, mt, :], pt[:, :w])

            def normalize_seg(hp, fb):
                # normalization of one (pair, F-block) segment; no
                # TensorEngine or PSUM involvement: DMA gather the psum-
                # produced denominators (already staged in ctxs row 64),
                # DVE reciprocal, DMA scatter to a row, gpsimd broadcast,
                # DVE multiply, DMA out.
                nfb = FB // P              # denom cols per head (8)
                den = fin.tile([P, 2 * nfb], dt.float32, tag="den", bufs=2,
                               name="den")
                for hip in range(2):
                    h = 2 * hp + hip
                    nc.sync.dma_start(
                        den[:, hip * nfb:(hip + 1) * nfb],
                        ctxs[DH:DH + 1, h, fb * FB:(fb + 1) * FB])
                rec = fin.tile([P, 2 * nfb], dt.float32, tag="rec", bufs=2,
                               name="rec")
                nc.vector.reciprocal(rec[:], den[:])
                for hip in range(2):
                    h = 2 * hp + hip
                    rr = fin.tile([1, FB], dt.float32, tag="recrow", bufs=2,
                                  name="rr")
                    nc.sync.dma_start(
                        rr[:], rec[:, hip * nfb:(hip + 1) * nfb])
                    rrb = fin.tile([DH, FB], dt.float32, tag="rrb", bufs=2,
                                   name="rrb")
                    nc.gpsimd.partition_broadcast(rrb[:], rr[:])
                    for n in range(NSUB):
                        fcol = fb * FB + n * 512
                        ot = fin.tile([DH, 512], dt.float32, tag="ot",
                                      bufs=3, name="ot")
                        nc.vector.tensor_tensor(
                            ot[:], ctxs[0:DH, h, fcol:fcol + 512],
                            rrb[:, n * 512:(n + 1) * 512], Alu.mult)
                        nc.sync.dma_start(
                            out.ap()[h * DH:(h + 1) * DH, fcol:fcol + 512],
                            ot[:])

            def fine_norm(hp, fb, ctx_ps):
                # last-segment epilogue+normalization in 512-wide chains:
                # each chunk's copy -> denom gather -> reciprocal ->
                # scatter -> broadcast -> multiply -> store overlaps the
                # neighbouring chunks on other engines.
                for n in range(NSUB):
                    for hip in range(2):
                        h = 2 * hp + hip
                        fcol = fb * FB + n * 512
                        nc.vector.tensor_copy(
                            ctxs[:, h, fcol:fcol + 512],
                            ctx_ps[hip][:, n * 512:(n + 1) * 512])
                        dnc = fin.tile([P, 4], dt.float32, tag="dnc",
                                       bufs=4, name="dnc")
                        nc.sync.dma_start(dnc[:],
                                          ctxs[DH:DH + 1, h, fcol:fcol + 512])
                        rcc = fin.tile([P, 4], dt.float32, tag="rcc",
                                       bufs=4, name="rcc")
                        nc.vector.reciprocal(rcc[:], dnc[:])
                        rrc = fin.tile([1, 512], dt.float32, tag="rrc",
                                       bufs=4, name="rrc")
                        nc.sync.dma_start(rrc[:], rcc[:])
                        rrbc = fin.tile([DH, 512], dt.float32, tag="rrbc",
                                        bufs=4, name="rrbc")
                        nc.gpsimd.partition_broadcast(rrbc[:], rrc[:])
                        otc = fin.tile([DH, 512], dt.float32, tag="ot",
                                       bufs=3, name="otc")
                        nc.vector.tensor_tensor(
                            otc[:], ctxs[0:DH, h, fcol:fcol + 512],
                            rrbc[:], Alu.mult)
                        nc.sync.dma_start(
                            out.ap()[h * DH:(h + 1) * DH, fcol:fcol + 512],
                            otc[:])

            def attn_pair(hp, mid_cb=None):
                # inner loops for head pair (2hp, 2hp+1); per-ti emission
                # [sA, ctxA(prev), expA, sB, ctxB(prev), expB]: each head's
                # next scores only gate on ITS OWN previous exp (psum slot
                # rotation), so ACT runs back-to-back.
                mtile = hp
                for fb in range(NFB):
                    if fb == 1 and mid_cb is not None:
                        mid_cb()
                    ctx_ps = [
                        ps_c.tile([DH + 1, FB], dt.float32, tag="c",
                                  name="ctx_ps")
                        for _ in range(2)
                    ]
                    exp_tiles = {}

                    def mm_ctx(hip, ti):
                        h = 2 * hp + hip
                        et = exp_tiles.pop((hip, ti))
                        for n in range(NSUB):
                            nc.tensor.matmul(
                                ctx_ps[hip][:, n * 512:(n + 1) * 512],
                                vp[:, ti, h * (DH + 1):(h + 1) * (DH + 1)],
                                et[:, n * 512:(n + 1) * 512],
                                start=(ti == 0), stop=(ti == NT - 1))

                    for ti in range(NT):
                        for hip in range(2):
                            off = hip * DH
                            s_ps = ps_s.tile([P, FB], dt.float32, tag="s",
                                             name="s_ps")
                            for n in range(NSUB):
                                fcol = fb * FB + n * 512
                                nc.tensor.matmul(
                                    s_ps[:, n * 512:(n + 1) * 512],
                                    kt[off:off + DH, mtile,
                                       ti * P:(ti + 1) * P],
                                    qt[off:off + DH, mtile, fcol:fcol + 512],
                                    start=True, stop=True)
                            if ti > 0:
                                mm_ctx(hip, ti - 1)
                            et = work.tile([P, FB], dt.bfloat16, tag="exp",
                                           name="et")
                            nc.scalar.activation(
                                et[:], s_ps[:], Exp,
                                bias=adder_sb[:, ti:ti + 1], scale=1.0)
                            exp_tiles[(hip, ti)] = et
                    mm_ctx(0, NT - 1)
                    mm_ctx(1, NT - 1)

                    # drain ctx' psum to sbuf staging, then normalize this
                    # segment (all off-PE; overlaps the next segment/pair).
                    # The very last segment uses finely-chunked chains so
                    # the kernel tail pipelines across DVE/DMA/GpSimd.
                    # epilogue copies; in the very last segment ACT is
                    # permanently idle, so run head B's copy there to
                    # parallelize the tail chain head with head A's on DVE.
                    last = (hp == 2 and fb == NFB - 1)
                    for hip in range(2):
                        h = 2 * hp + hip
                        dst = ctxs[:, h, fb * FB:(fb + 1) * FB]
                        if last and hip == 1:
                            nc.scalar.copy(dst, ctx_ps[hip][:])
                        else:
                            nc.vector.tensor_copy(dst, ctx_ps[hip][:])
                    normalize_seg(hp, fb)

            # emission order: minimal prefix (qt/kt m0 + pair-0 V), then
            # pair-0 attention; later projections are boundary blocks that
            # partially hide behind the previous pair's exp backlog.
            # PE warm-up: garbage matmuls with no input deps run during the
            # initial DMA wait, releasing the HAM clock throttle before the
            # real projections start.
            warm = sb.tile([P, 512], dt.bfloat16, tag="warm")
            nc.gpsimd.memset(warm[:], 0.0)
            wexp = sb.tile([P, 1], dt.bfloat16, tag="wexp")
            nc.scalar.activation(wexp[:], warm[:, 0:1], Exp)
            wpt = ps_s.tile([P, 512], dt.float32, tag="s", name="wpt")
            for wi in range(20):
                nc.tensor.matmul(wpt[:], warm[:, 0:P], warm[:],
                                 start=(wi == 0), stop=(wi == 19))
            wpt2 = ps_s.tile([P, 512], dt.float32, tag="s", name="wpt2")
            for wi in range(28):
                nc.tensor.matmul(wpt2[:, 0:256], warm[:, 0:P],
                                 warm[:, 0:256],
                                 start=(wi == 0), stop=(wi == 27))

            proj_qk(wqs, qt, 0)
            proj_qk(wks, kt, 0)
            for mt in range(NT):
                proj_v(mt)
            with tc.high_priority():
                attn_pair(0)
            proj_qk(wqs, qt, 1)
            proj_qk(wks, kt, 1)
            with tc.high_priority():
                attn_pair(1)
            proj_qk(wqs, qt, 2)
            proj_qk(wks, kt, 2)
            with tc.high_priority():
                attn_pair(2)

    nc.compile()
    return nc


def _prep_core_inputs(c, x, Wq, bq, Wk, bk, Wv, bv, mask):
    bf16 = ml_dtypes.bfloat16
    b, hg = c // 2, c % 2
    cols = slice(hg * DL, (hg + 1) * DL)

    xT_aug = np.empty((D + 1, S), dtype=bf16)
    xT_aug[:D] = x[b].T.astype(bf16)
    xT_aug[D] = np.float32(1.0)

    wq_aug = np.empty((D + 1, DL), dtype=bf16)
    wq_aug[:D] = (Wq[:, cols] / 8.0).astype(bf16)
    wq_aug[D] = (bq[cols] / 8.0).astype(bf16)

    wk_aug = np.empty((D + 1, DL), dtype=bf16)
    wk_aug[:D] = Wk[:, cols].astype(bf16)
    wk_aug[D] = bk[cols].astype(bf16)

    wv_aug = np.zeros((D + 1, HL * (DH + 1)), dtype=bf16)
    wv_loc = Wv[:, cols].astype(np.float32)
    bv_loc = bv[cols].astype(np.float32)
    for j in range(HL):
        wv_aug[:D, j * (DH + 1):j * (DH + 1) + DH] = \
            wv_loc[:, j * DH:(j + 1) * DH].astype(bf16)
        wv_aug[D, j * (DH + 1):j * (DH + 1) + DH] = \
            bv_loc[j * DH:(j + 1) * DH].astype(bf16)
        wv_aug[D, j * (DH + 1) + DH] = np.float32(1.0)

    add = ((mask[b].astype(np.float32) - 1.0) * 10000.0)
    adder_t = add.reshape(NT, P).T.copy()   # [128,16]: [p, ti] = add[ti*128+p]

    return {"xT": xT_aug, "wq": wq_aug, "wk": wk_aug, "wv": wv_aug,
            "adder": np.ascontiguousarray(adder_t, dtype=np.float32)}


def kernel(x, Wq, bq, Wk, bk, Wv, bv, mask, _trace=False):
    from concourse.bass_utils import run_bass_kernel_spmd

    x = np.asarray(x, dtype=np.float32)
    Wq = np.asarray(Wq, dtype=np.float32)
    bq = np.asarray(bq, dtype=np.float32)
    Wk = np.asarray(Wk, dtype=np.float32)
    bk = np.asarray(bk, dtype=np.float32)
    Wv = np.asarray(Wv, dtype=np.float32)
    bv = np.asarray(bv, dtype=np.float32)
    mask = np.asarray(mask)

    with_bias = bool(bq.any() or bk.any() or bv.any())
    key = ("nc", with_bias)
    if key not in _CACHE:
        _CACHE[key] = _build(with_bias=with_bias)
    nc = _CACHE[key]

    in_maps = [_prep_core_inputs(c, x, Wq, bq, Wk, bk, Wv, bv, mask)
               for c in range(NCORES)]
    res = run_bass_kernel_spmd(nc, in_maps, core_ids=list(range(NCORES)),
                               trace=_trace)
    if _trace:
        _CACHE["last_result"] = res

    full = np.empty((B, S, D), dtype=np.float32)
    for c in range(NCORES):
        b, hg = c // 2, c % 2
        full[b, :, hg * DL:(hg + 1) * DL] = res.results[c]["out"].T
    return full

